# revision 30
# baseline (speedup 1.0000x reference)
"""PWC-Net local correlation (MD=4, 81 displacements) on 8 Trainium2 cores.

Problem: t1, t2: [B=4, C=128, H=128, W=256] fp32
  out[b, d, y, x] = mean_c t1[b,c,y,x] * t2pad[b,c,y+dy,x+dx],  d = (dy+4)*9+(dx+4)

Sharding: 8 cores = B(4) x W-half(2); inputs pre-sliced/padded/bf16-cast on
host (t1 pre-scaled by 1/C so the gram is already the mean).

v5: 2D-patch grams (16x8 pixels) instead of column grams -- 3.2x less PE and
evacuation work per pixel. Per patch (yb, xb): one bf16 matmul
  G[p, j] = sum_c t1[c, p] * t2p[c, window j],  p = yl*8+cc (128 pixels),
  j = wr*16 + wc over the 24x16 halo window (384 cols, one PSUM bank).
Useful entries: out[p, dy, dx] = G[p, 16*yl + cc + 16*dy + dx].

Four x-adjacent patches (xb = 4*xg+lane) form a qgroup. Hard-won structural
constraints (this + prior session evidence):
  - Tile serializes two readers of one PSUM tile cross-engine (ACT->DVE
    chain), and two writers of one SBUF tile -- so lanes 0,1 go to psP
    (read by ACT only) and lanes 2,3 to psQ (DVE only), each engine
    pair-interleaving its half into its own gsb tile:
    gsbA[p, slot*768 + j*2 + lane] (ACT), gsbB same (DVE).
  - composite AP dims crossing partitions+columns are illegal ("illegal
    partition step"), so per-octet rebased dumps are impossible; dumps move
    partition-uniform windows per 16-partition group gi (yl pair):
    j in [32*gi, 32*gi+160), flat cols [64*gi, 64*gi+320) of every slot,
    640B-contiguous runs, straight to output DRAM (1.98x amplification).
    Host does the final 81-of-640 gather (junk discard + (d,y,x) transpose);
    all arithmetic stays on device.
  - A-half dumps ride HWDGE (632ns/instr, shared); B-half dumps ride the
    otherwise-idle Pool/SWDGE path as gi-PAIR windows (32 partitions,
    [128g, +384), amp 2.37) -- 4 instead of 8 SWDGE issues per supertile so
    the last supertile's ~1us/instr Pool drain doesn't tail the run.
  - matmul PSUM outputs must stay inside one 2KB bank (512-aligned chunks);
    a strided interleaving matmul out AP silently corrupts.
  - matmul stationary (lhsT) APs allow one free dim -> t1 is pre-arranged
    patch-major on host so each patch's 128 pixels are contiguous.
  - DRAM->DRAM skewed DMA: fatal on hardware (NRT_EXEC_UNIT_UNRECOVERABLE).
  - t2 y-halo rows are NOT re-loaded from DRAM: slab q rows [0,8) come from
    slab q-1 rows [32,40) via a DVE SBUF->SBUF bf16 copy; all-pad rows are
    memset on-chip. Saves ~1.4MB HBM per core.
Steady state is DMA_ENGINES-bound: ~14.4MB/core of HBM traffic (8.7 in,
5.8 out at ~2x junk amplification -- the price of partition-uniform dump
windows) against ~360GB/s. Engines sit at 50-65%: ACT 28us, DVE 32us,
PE 24us, Pool 18us, HWDGE 26us.
v1 (column grams): 73.3us HW / 141us cost model. v5: 47.6us cost model.
"""

import numpy as np
import ml_dtypes

B, C, H, W = 4, 128, 128, 256
MD = 4
D = (2 * MD + 1) ** 2  # 81
WH = W // 2  # 128 columns per core
YP2 = H + 2 * MD  # 136 padded t2 rows
XP2 = WH + 2 * MD  # 136 padded t2 cols
WIN = 384  # 24x16 window cols per patch gram
LANES = 4  # patches per qgroup (2 per engine half)
HSLOT = 2 * WIN  # 768 gsb cols per slot in each half tile
NSLOT = 8  # slots per supertile (= 2 y-blocks x 4 x-groups)
GWH = 320  # per-gi dump window: 160 j x 2 lanes
GWB = 384  # gi-pair dump window: 192 j x 2 lanes
ASZ = 4 * 8 * 16 * NSLOT * GWH  # A region (unpaired gi dumps)
BSZ = 4 * 4 * 32 * NSLOT * GWB  # B region (gi-paired dumps: short Pool tail)
_compiled = None


def _build(reps=None):
    """Build the per-core program. reps=None: single pass. reps=R: wrap the
    compute in a hardware For loop (identical output each rep) for wall-clock
    benchmarking through the noisy RPC dispatch floor."""
    import concourse.bacc as bacc
    import concourse.bass as bass
    import concourse.mybir as mybir
    import concourse.tile as tile

    bf = mybir.dt.bfloat16
    nc = bacc.Bacc("TRN2", target_bir_lowering=False, debug=False, num_devices=8)
    t1s = nc.dram_tensor("t1s", [C, H * WH], bf, kind="ExternalInput").ap()
    t2s = nc.dram_tensor("t2s", [C, 4 * 40 * XP2], bf, kind="ExternalInput").ap()
    outp = nc.dram_tensor("outp", [ASZ + BSZ], bf, kind="ExternalOutput").ap()

    with tile.TileContext(nc) as tc:
        with (
            tc.tile_pool(name="inputs", bufs=1) as inp,
            tc.tile_pool(name="gpool", bufs=4) as gpool,
            tc.tile_pool(name="psA", bufs=2, space="PSUM") as ppa,
            tc.tile_pool(name="psB", bufs=2, space="PSUM") as ppb,
        ):
            # one slab per supertile (yb pair): t1 4096 px patch-major,
            # t2 rows [32q-4, 32q+36) pre-padded on host
            t1t = [inp.tile([C, 32 * WH], bf, name=f"t1t_{q}") for q in range(4)]
            t2t = [inp.tile([C, 40 * XP2], bf, name=f"t2t_{q}") for q in range(4)]
            S1 = t1t[0].tensor.shape[-1]
            S2 = t2t[0].tensor.shape[-1]

            def load_slab(q, split):
                """Issue slab q's loads; split=True stages the pieces the
                first qgroups need in front (t1 1024-px mini, t2 24 rows).
                Slabs q>=1 load only rows [8,40) from DRAM; rows [0,8) (the
                8-row halo, = slab q-1's rows [32,40)) are filled by a cheap
                DVE SBUF->SBUF copy (bf16 4x mode), saving 0.84MB of HBM."""
                if split:
                    # rows [0,4) of slab 0 are all-pad: memset, load [4,24)
                    nc.vector.memset(
                        bass.AP(t2t[q].tensor, 0, [[S2, C], [1, 4 * XP2]]), 0.0
                    )
                    nc.sync.dma_start(
                        bass.AP(t1t[q].tensor, 0, [[S1, C], [1, 1024]]),
                        bass.AP(t1s.tensor, 32 * WH * q, [[H * WH, C], [1, 1024]]),
                    )
                    nc.sync.dma_start(
                        bass.AP(t2t[q].tensor, 4 * XP2, [[S2, C], [1, 20 * XP2]]),
                        bass.AP(t2s.tensor, 40 * XP2 * q + 4 * XP2,
                                [[4 * 40 * XP2, C], [1, 20 * XP2]]),
                    )
                    nc.sync.dma_start(
                        bass.AP(t1t[q].tensor, 1024, [[S1, C], [1, 32 * WH - 1024]]),
                        bass.AP(t1s.tensor, 32 * WH * q + 1024,
                                [[H * WH, C], [1, 32 * WH - 1024]]),
                    )
                    nc.sync.dma_start(
                        bass.AP(t2t[q].tensor, 24 * XP2, [[S2, C], [1, 16 * XP2]]),
                        bass.AP(t2s.tensor, 40 * XP2 * q + 24 * XP2,
                                [[4 * 40 * XP2, C], [1, 16 * XP2]]),
                    )
                else:
                    nc.sync.dma_start(
                        t1t[q][:],
                        bass.AP(t1s.tensor, 32 * WH * q, [[H * WH, C], [1, 32 * WH]]),
                    )
                    # rows [36,40) of slab 3 are all-pad: memset instead
                    rows = 28 if q == 3 else 32
                    nc.sync.dma_start(
                        bass.AP(t2t[q].tensor, 8 * XP2, [[S2, C], [1, rows * XP2]]),
                        bass.AP(t2s.tensor, 40 * XP2 * q + 8 * XP2,
                                [[4 * 40 * XP2, C], [1, rows * XP2]]),
                    )
                    if q == 3:
                        nc.vector.memset(
                            bass.AP(t2t[q].tensor, 36 * XP2, [[S2, C], [1, 4 * XP2]]),
                            0.0,
                        )
                    nc.vector.tensor_copy(
                        bass.AP(t2t[q].tensor, 0, [[S2, C], [1, 8 * XP2]]),
                        bass.AP(t2t[q - 1].tensor, 32 * XP2, [[S2, C], [1, 8 * XP2]]),
                    )

            load_slab(0, split=True)
            load_slab(1, split=False)

            def body(_iv=None):
                for st in range(4):
                    if st + 2 < 4:
                        # prefetch slab st+2 AFTER the previous supertile's
                        # dumps in SP program order, so dump transfers aren't
                        # starved on DMA_ENGINES by bulk input loads
                        load_slab(st + 2, split=False)
                    gsbA = gpool.tile([C, NSLOT * HSLOT], bf, name="gsbA")
                    gsbB = gpool.tile([C, NSLOT * HSLOT], bf, name="gsbB")
                    SgA = gsbA.tensor.shape[-1]
                    SgB = gsbB.tensor.shape[-1]
                    for sl in range(NSLOT):
                        ybh, xg = sl // 4, sl % 4
                        psP = ppa.tile([128, 1024], mybir.dt.float32, name="psP")
                        psQ = ppb.tile([128, 1024], mybir.dt.float32, name="psQ")
                        SpP = psP.tensor.shape[-1]
                        SpQ = psQ.tensor.shape[-1]
                        for lane in range(LANES):
                            xb = LANES * xg + lane
                            lhsT = bass.AP(
                                t1t[st].tensor, (ybh * 16 + xb) * 128,
                                [[S1, C], [1, 128]],
                            )
                            rhs = bass.AP(
                                t2t[st].tensor, (16 * ybh) * XP2 + 8 * xb,
                                [[S2, C], [XP2, 24], [1, 16]],
                            )
                            pst, Spt = (psP, SpP) if lane < 2 else (psQ, SpQ)
                            nc.tensor.matmul(
                                bass.AP(pst.tensor, 512 * (lane % 2),
                                        [[Spt, 128], [1, WIN]]),
                                lhsT, rhs, start=True, stop=True,
                            )
                        # pair-interleaving evacuation per engine half:
                        # gsbX[p, sl*768 + j*2 + lane%2]
                        nc.scalar.copy(
                            bass.AP(gsbA.tensor, HSLOT * sl,
                                    [[SgA, 128], [2, WIN], [1, 2]]),
                            bass.AP(psP.tensor, 0, [[SpP, 128], [1, WIN], [512, 2]]),
                        )
                        nc.vector.tensor_copy(
                            bass.AP(gsbB.tensor, HSLOT * sl,
                                    [[SgB, 128], [2, WIN], [1, 2]]),
                            bass.AP(psQ.tensor, 0, [[SpQ, 128], [1, WIN], [512, 2]]),
                        )
                    # dumps. A-half on HWDGE (SP): per 16-partition group gi
                    # (yl pair), window j in [32gi, 32gi+160) = cols
                    # [64gi, +320) per slot. B-half on Pool/SWDGE: 32-partition
                    # gi-PAIRS (window [128g, +384)) -- 4 instead of 8 ~1us
                    # SWDGE issues per supertile; unpaired-B measured slower
                    # (49.5us) despite fewer bytes because Pool issue rate
                    # starved the B transfers mid-run and tailed the end.
                    for gi in range(8):
                        nc.sync.dma_start(
                            bass.AP(outp.tensor,
                                    st * (8 * 16 * NSLOT * GWH)
                                    + gi * (16 * NSLOT * GWH),
                                    [[NSLOT * GWH, 16], [GWH, NSLOT], [1, GWH]]),
                            bass.AP(gsbA.tensor, (16 * gi) * SgA + 64 * gi,
                                    [[SgA, 16], [HSLOT, NSLOT], [1, GWH]]),
                        )
                    for g in range(4):
                        nc.gpsimd.dma_start(
                            bass.AP(outp.tensor,
                                    ASZ + st * (4 * 32 * NSLOT * GWB)
                                    + g * (32 * NSLOT * GWB),
                                    [[NSLOT * GWB, 32], [GWB, NSLOT], [1, GWB]]),
                            bass.AP(gsbB.tensor, (32 * g) * SgB + 128 * g,
                                    [[SgB, 32], [HSLOT, NSLOT], [1, GWB]]),
                        )

            if reps is None:
                body()
            else:
                with tc.For_i(0, reps, 1) as iv:
                    body(iv)

    nc.compile()
    return nc


def _prep_inputs(t1, t2):
    bf16 = ml_dtypes.bfloat16
    in_maps = []
    for k in range(8):
        b, xh = k // 2, k % 2
        xs = xh * WH
        t1c = (t1[b, :, :, xs : xs + WH] * (1.0 / C)).astype(bf16)
        # patch-major: [c, ((yb*16+xb)*16 + yl)*8 + cc], yb in 0..7, xb in
        # 0..15 (matmul stationary APs allow one free dim only)
        t1blk = np.ascontiguousarray(
            t1c.reshape(C, 8, 16, 16, 8).transpose(0, 1, 3, 2, 4)
        ).reshape(C, H * WH)
        t2p = np.zeros((C, YP2, XP2), dtype=bf16)
        lo, hi = max(0, xs - MD), min(W, xs + WH + MD)
        t2p[:, MD : MD + H, lo - (xs - MD) : hi - (xs - MD)] = t2[b, :, :, lo:hi].astype(bf16)
        # overlapping 40-row slabs per supertile (yb pair)
        t2blk = np.concatenate(
            [t2p[:, 32 * q : 32 * q + 40, :].reshape(C, 40 * XP2) for q in range(4)],
            axis=1,
        )
        in_maps.append({"t1s": t1blk, "t2s": t2blk})
    return in_maps


# host gathers. A: R_A[pq16, dy, dx, l] = (16*(pq//8) + pq%8 + 16dy + dx)*2 + l
# B: R_B[pq32, dy, dx, l] same formula with pq32//8 in 0..3 (gi-pair groups).
def _mk_R(npq):
    pq = np.arange(npq)
    base = 16 * (pq // 8) + (pq % 8)
    return (
        (base[:, None, None, None]
         + 16 * np.arange(9)[None, :, None, None]
         + np.arange(9)[None, None, :, None]) * 2
        + np.arange(2)[None, None, None, :]
    ).reshape(1, 1, npq, 1, 162)


_RA = _mk_R(16)
_RB = _mk_R(32)


def kernel(t1: np.ndarray, t2: np.ndarray) -> np.ndarray:
    from concourse.bass_utils import run_bass_kernel_spmd

    global _compiled
    if _compiled is None:
        _compiled = _build()
    nc = _compiled

    t1 = np.asarray(t1, dtype=np.float32)
    t2 = np.asarray(t2, dtype=np.float32)
    res = run_bass_kernel_spmd(nc, _prep_inputs(t1, t2), list(range(8)))

    out = np.empty((B, D, H, W), dtype=np.float32)
    for k in range(8):
        b, xh = k // 2, k % 2
        xs = xh * WH
        raw = res.results[k]["outp"].astype(np.float32)
        # x = 32*slx + 8*lane + cc ; y = 32*st + 16*slh + (yl)
        oc = np.empty((D, H, 4, 4, 8), dtype=np.float32)
        arrA = raw[:ASZ].reshape(4, 8, 16, NSLOT, GWH)
        gA = np.take_along_axis(arrA, _RA, axis=4)  # [st,gi,pq,slot,(dy,dx,l)]
        gA = gA.reshape(4, 8, 2, 8, 2, 4, 9, 9, 2)  # [st,gi,ylq,cc,slh,slx,dy,dx,l]
        # y = 32st + 16slh + 2gi + ylq
        gA = gA.transpose(6, 7, 0, 4, 1, 2, 5, 8, 3)  # [dy,dx,st,slh,gi,ylq,slx,l,cc]
        oc[:, :, :, 0:2, :] = gA.reshape(D, H, 4, 2, 8)
        arrB = raw[ASZ:].reshape(4, 4, 32, NSLOT, GWB)
        gB = np.take_along_axis(arrB, _RB, axis=4)  # [st,g,pq32,slot,(dy,dx,l)]
        gB = gB.reshape(4, 4, 4, 8, 2, 4, 9, 9, 2)  # [st,g,ylq4,cc,slh,slx,dy,dx,l]
        # y = 32st + 16slh + 4g + ylq4
        gB = gB.transpose(6, 7, 0, 4, 1, 2, 5, 8, 3)
        oc[:, :, :, 2:4, :] = gB.reshape(D, H, 4, 2, 8)
        out[b, :, :, xs : xs + WH] = oc.reshape(D, H, WH)
    return out


# revision 36
# speedup vs baseline: 1.0015x; 1.0015x over previous
"""PWC-Net local correlation (MD=4, 81 displacements) on 8 Trainium2 cores.

Problem: t1, t2: [B=4, C=128, H=128, W=256] fp32
  out[b, d, y, x] = mean_c t1[b,c,y,x] * t2pad[b,c,y+dy,x+dx],  d = (dy+4)*9+(dx+4)

Sharding: 8 cores = B(4) x W-half(2); inputs pre-sliced/padded/bf16-cast on
host (t1 pre-scaled by 1/C so the gram is already the mean).

v5: 2D-patch grams (16x8 pixels) instead of column grams -- 3.2x less PE and
evacuation work per pixel. Per patch (yb, xb): one bf16 matmul
  G[p, j] = sum_c t1[c, p] * t2p[c, window j],  p = yl*8+cc (128 pixels),
  j = wr*16 + wc over the 24x16 halo window (384 cols, one PSUM bank).
Useful entries: out[p, dy, dx] = G[p, 16*yl + cc + 16*dy + dx].

Four x-adjacent patches (xb = 4*xg+lane) form a qgroup. Hard-won structural
constraints (this + prior session evidence):
  - Tile serializes two readers of one PSUM tile cross-engine (ACT->DVE
    chain), and two writers of one SBUF tile -- so lanes 0,1 go to psP
    (read by ACT only) and lanes 2,3 to psQ (DVE only), each engine
    pair-interleaving its half into its own gsb tile:
    gsbA[p, slot*768 + j*2 + lane] (ACT), gsbB same (DVE).
  - composite AP dims crossing partitions+columns are illegal ("illegal
    partition step"), so per-octet rebased dumps are impossible; dumps move
    partition-uniform windows per 16-partition group gi (yl pair):
    j in [32*gi, 32*gi+160), flat cols [64*gi, 64*gi+320) of every slot,
    640B-contiguous runs, straight to output DRAM (1.98x amplification).
    Host does the final 81-of-640 gather (junk discard + (d,y,x) transpose);
    all arithmetic stays on device.
  - A-half dumps ride HWDGE (632ns/instr, shared); B-half dumps ride the
    otherwise-idle Pool/SWDGE path as gi-PAIR windows (32 partitions,
    [128g, +384), amp 2.37) -- 4 instead of 8 SWDGE issues per supertile so
    the last supertile's ~1us/instr Pool drain doesn't tail the run.
  - matmul PSUM outputs must stay inside one 2KB bank (512-aligned chunks);
    a strided interleaving matmul out AP silently corrupts.
  - matmul stationary (lhsT) APs allow one free dim -> t1 is pre-arranged
    patch-major on host so each patch's 128 pixels are contiguous.
  - DRAM->DRAM skewed DMA: fatal on hardware (NRT_EXEC_UNIT_UNRECOVERABLE).
  - t2 y-halo rows are NOT re-loaded from DRAM: slab q rows [0,8) come from
    slab q-1 rows [32,40) via a DVE SBUF->SBUF bf16 copy; all-pad rows are
    memset on-chip. Saves ~1.4MB HBM per core.
Steady state is DMA_ENGINES-bound: ~14.4MB/core of HBM traffic (8.7 in,
5.8 out at ~2x junk amplification -- the price of partition-uniform dump
windows) against ~360GB/s. Engines sit at 50-65%: ACT 28us, DVE 32us,
PE 24us, Pool 18us, HWDGE 26us.
v1 (column grams): 73.3us HW / 141us cost model. v5: 47.6us cost model.
"""

import numpy as np
import ml_dtypes

B, C, H, W = 4, 128, 128, 256
MD = 4
D = (2 * MD + 1) ** 2  # 81
WH = W // 2  # 128 columns per core
YP2 = H + 2 * MD  # 136 padded t2 rows
XP2 = WH + 2 * MD  # 136 padded t2 cols
WIN = 384  # 24x16 window cols per patch gram
LANES = 4  # patches per qgroup (2 per engine half)
HSLOT = 2 * WIN  # 768 gsb cols per slot in each half tile
NSLOT = 8  # slots per supertile (= 2 y-blocks x 4 x-groups)
GWH = 320  # per-gi dump window: 160 j x 2 lanes
GWB = 384  # gi-pair dump window: 192 j x 2 lanes
GW4 = 512  # 64-partition (4-gi) dump window: 256 j x 2 lanes
ASZ = 3 * 8 * 16 * NSLOT * GWH  # A region st 0-2 (unpaired gi dumps)
BSZ = 3 * 4 * 32 * NSLOT * GWB  # B region st 0-2 (gi-paired dumps)
A3SZ = 4 * 32 * NSLOT * GWB  # A region st 3: gi-paired, 4 HWDGE issues
B3SZ = 2 * 64 * NSLOT * GW4  # B region st 3: 2 Pool issues -> short tail
_compiled = None


def _build(reps=None):
    """Build the per-core program. reps=None: single pass. reps=R: wrap the
    compute in a hardware For loop (identical output each rep) for wall-clock
    benchmarking through the noisy RPC dispatch floor."""
    import concourse.bacc as bacc
    import concourse.bass as bass
    import concourse.mybir as mybir
    import concourse.tile as tile

    bf = mybir.dt.bfloat16
    nc = bacc.Bacc("TRN2", target_bir_lowering=False, debug=False, num_devices=8)
    t1s = nc.dram_tensor("t1s", [C, H * WH], bf, kind="ExternalInput").ap()
    t2s = nc.dram_tensor("t2s", [C, 4 * 40 * XP2], bf, kind="ExternalInput").ap()
    outp = nc.dram_tensor("outp", [ASZ + BSZ + A3SZ + B3SZ], bf,
                          kind="ExternalOutput").ap()

    with tile.TileContext(nc) as tc:
        with (
            tc.tile_pool(name="inputs", bufs=1) as inp,
            tc.tile_pool(name="gpool", bufs=4) as gpool,
            tc.tile_pool(name="psA", bufs=2, space="PSUM") as ppa,
            tc.tile_pool(name="psB", bufs=2, space="PSUM") as ppb,
        ):
            # one slab per supertile (yb pair): t1 4096 px patch-major,
            # t2 rows [32q-4, 32q+36) pre-padded on host
            t1t = [inp.tile([C, 32 * WH], bf, name=f"t1t_{q}") for q in range(4)]
            t2t = [inp.tile([C, 40 * XP2], bf, name=f"t2t_{q}") for q in range(4)]
            S1 = t1t[0].tensor.shape[-1]
            S2 = t2t[0].tensor.shape[-1]

            def load_slab(q, split):
                """Issue slab q's loads; split=True stages the pieces the
                first qgroups need in front (t1 1024-px mini, t2 24 rows).
                Slabs q>=1 load only rows [8,40) from DRAM; rows [0,8) (the
                8-row halo, = slab q-1's rows [32,40)) are filled by a cheap
                DVE SBUF->SBUF copy (bf16 4x mode), saving 0.84MB of HBM."""
                if split:
                    # rows [0,4) of slab 0 are all-pad: memset, load [4,24)
                    nc.vector.memset(
                        bass.AP(t2t[q].tensor, 0, [[S2, C], [1, 4 * XP2]]), 0.0
                    )
                    nc.sync.dma_start(
                        bass.AP(t1t[q].tensor, 0, [[S1, C], [1, 1024]]),
                        bass.AP(t1s.tensor, 32 * WH * q, [[H * WH, C], [1, 1024]]),
                    )
                    nc.sync.dma_start(
                        bass.AP(t2t[q].tensor, 4 * XP2, [[S2, C], [1, 20 * XP2]]),
                        bass.AP(t2s.tensor, 40 * XP2 * q + 4 * XP2,
                                [[4 * 40 * XP2, C], [1, 20 * XP2]]),
                    )
                    nc.sync.dma_start(
                        bass.AP(t1t[q].tensor, 1024, [[S1, C], [1, 32 * WH - 1024]]),
                        bass.AP(t1s.tensor, 32 * WH * q + 1024,
                                [[H * WH, C], [1, 32 * WH - 1024]]),
                    )
                    nc.sync.dma_start(
                        bass.AP(t2t[q].tensor, 24 * XP2, [[S2, C], [1, 16 * XP2]]),
                        bass.AP(t2s.tensor, 40 * XP2 * q + 24 * XP2,
                                [[4 * 40 * XP2, C], [1, 16 * XP2]]),
                    )
                else:
                    nc.sync.dma_start(
                        t1t[q][:],
                        bass.AP(t1s.tensor, 32 * WH * q, [[H * WH, C], [1, 32 * WH]]),
                    )
                    # rows [36,40) of slab 3 are all-pad: memset instead
                    rows = 28 if q == 3 else 32
                    nc.sync.dma_start(
                        bass.AP(t2t[q].tensor, 8 * XP2, [[S2, C], [1, rows * XP2]]),
                        bass.AP(t2s.tensor, 40 * XP2 * q + 8 * XP2,
                                [[4 * 40 * XP2, C], [1, rows * XP2]]),
                    )
                    if q == 3:
                        nc.vector.memset(
                            bass.AP(t2t[q].tensor, 36 * XP2, [[S2, C], [1, 4 * XP2]]),
                            0.0,
                        )
                    nc.vector.tensor_copy(
                        bass.AP(t2t[q].tensor, 0, [[S2, C], [1, 8 * XP2]]),
                        bass.AP(t2t[q - 1].tensor, 32 * XP2, [[S2, C], [1, 8 * XP2]]),
                    )

            load_slab(0, split=True)
            load_slab(1, split=False)

            def body(_iv=None):
                for st in range(4):
                    if st + 2 < 4:
                        # prefetch slab st+2 AFTER the previous supertile's
                        # dumps in SP program order, so dump transfers aren't
                        # starved on DMA_ENGINES by bulk input loads
                        load_slab(st + 2, split=False)
                    gsbA = gpool.tile([C, NSLOT * HSLOT], bf, name="gsbA")
                    gsbB = gpool.tile([C, NSLOT * HSLOT], bf, name="gsbB")
                    SgA = gsbA.tensor.shape[-1]
                    SgB = gsbB.tensor.shape[-1]
                    for sl in range(NSLOT):
                        ybh, xg = sl // 4, sl % 4
                        psP = ppa.tile([128, 1024], mybir.dt.float32, name="psP")
                        psQ = ppb.tile([128, 1024], mybir.dt.float32, name="psQ")
                        SpP = psP.tensor.shape[-1]
                        SpQ = psQ.tensor.shape[-1]
                        for lane in range(LANES):
                            xb = LANES * xg + lane
                            lhsT = bass.AP(
                                t1t[st].tensor, (ybh * 16 + xb) * 128,
                                [[S1, C], [1, 128]],
                            )
                            rhs = bass.AP(
                                t2t[st].tensor, (16 * ybh) * XP2 + 8 * xb,
                                [[S2, C], [XP2, 24], [1, 16]],
                            )
                            pst, Spt = (psP, SpP) if lane < 2 else (psQ, SpQ)
                            nc.tensor.matmul(
                                bass.AP(pst.tensor, 512 * (lane % 2),
                                        [[Spt, 128], [1, WIN]]),
                                lhsT, rhs, start=True, stop=True,
                            )
                        # pair-interleaving evacuation per engine half:
                        # gsbX[p, sl*768 + j*2 + lane%2]
                        nc.scalar.copy(
                            bass.AP(gsbA.tensor, HSLOT * sl,
                                    [[SgA, 128], [2, WIN], [1, 2]]),
                            bass.AP(psP.tensor, 0, [[SpP, 128], [1, WIN], [512, 2]]),
                        )
                        nc.vector.tensor_copy(
                            bass.AP(gsbB.tensor, HSLOT * sl,
                                    [[SgB, 128], [2, WIN], [1, 2]]),
                            bass.AP(psQ.tensor, 0, [[SpQ, 128], [1, WIN], [512, 2]]),
                        )
                    # dumps. A-half on HWDGE (SP): per 16-partition group gi
                    # (yl pair), window j in [32gi, 32gi+160) = cols
                    # [64gi, +320) per slot. B-half on Pool/SWDGE: 32-partition
                    # gi-PAIRS (window [128g, +384)) -- 4 instead of 8 ~1us
                    # SWDGE issues per supertile; unpaired-B measured slower
                    # (49.5us) despite fewer bytes because Pool issue rate
                    # starved the B transfers mid-run and tailed the end.
                    if st < 3:
                        for gi in range(8):
                            nc.sync.dma_start(
                                bass.AP(outp.tensor,
                                        st * (8 * 16 * NSLOT * GWH)
                                        + gi * (16 * NSLOT * GWH),
                                        [[NSLOT * GWH, 16], [GWH, NSLOT], [1, GWH]]),
                                bass.AP(gsbA.tensor, (16 * gi) * SgA + 64 * gi,
                                        [[SgA, 16], [HSLOT, NSLOT], [1, GWH]]),
                            )
                        for g in range(4):
                            nc.gpsimd.dma_start(
                                bass.AP(outp.tensor,
                                        ASZ + st * (4 * 32 * NSLOT * GWB)
                                        + g * (32 * NSLOT * GWB),
                                        [[NSLOT * GWB, 32], [GWB, NSLOT], [1, GWB]]),
                                bass.AP(gsbB.tensor, (32 * g) * SgB + 128 * g,
                                        [[SgB, 32], [HSLOT, NSLOT], [1, GWB]]),
                            )
                    else:
                        # last supertile: minimize issue-serialized drain --
                        # A paired (4 HWDGE), B as two 64-partition windows
                        # (2 Pool issues, j in [128g2, 128g2+272))
                        for g in range(4):
                            nc.sync.dma_start(
                                bass.AP(outp.tensor,
                                        ASZ + BSZ + g * (32 * NSLOT * GWB),
                                        [[NSLOT * GWB, 32], [GWB, NSLOT], [1, GWB]]),
                                bass.AP(gsbA.tensor, (32 * g) * SgA + 128 * g,
                                        [[SgA, 32], [HSLOT, NSLOT], [1, GWB]]),
                            )
                        for g2 in range(2):
                            nc.gpsimd.dma_start(
                                bass.AP(outp.tensor,
                                        ASZ + BSZ + A3SZ + g2 * (64 * NSLOT * GW4),
                                        [[NSLOT * GW4, 64], [GW4, NSLOT], [1, GW4]]),
                                bass.AP(gsbB.tensor, (64 * g2) * SgB + 256 * g2,
                                        [[SgB, 64], [HSLOT, NSLOT], [1, GW4]]),
                            )

            if reps is None:
                body()
            else:
                with tc.For_i(0, reps, 1) as iv:
                    body(iv)

    nc.compile()
    return nc


def _prep_inputs(t1, t2):
    bf16 = ml_dtypes.bfloat16
    in_maps = []
    for k in range(8):
        b, xh = k // 2, k % 2
        xs = xh * WH
        t1c = (t1[b, :, :, xs : xs + WH] * (1.0 / C)).astype(bf16)
        # patch-major: [c, ((yb*16+xb)*16 + yl)*8 + cc], yb in 0..7, xb in
        # 0..15 (matmul stationary APs allow one free dim only)
        t1blk = np.ascontiguousarray(
            t1c.reshape(C, 8, 16, 16, 8).transpose(0, 1, 3, 2, 4)
        ).reshape(C, H * WH)
        t2p = np.zeros((C, YP2, XP2), dtype=bf16)
        lo, hi = max(0, xs - MD), min(W, xs + WH + MD)
        t2p[:, MD : MD + H, lo - (xs - MD) : hi - (xs - MD)] = t2[b, :, :, lo:hi].astype(bf16)
        # overlapping 40-row slabs per supertile (yb pair)
        t2blk = np.concatenate(
            [t2p[:, 32 * q : 32 * q + 40, :].reshape(C, 40 * XP2) for q in range(4)],
            axis=1,
        )
        in_maps.append({"t1s": t1blk, "t2s": t2blk})
    return in_maps


# host gathers. A: R_A[pq16, dy, dx, l] = (16*(pq//8) + pq%8 + 16dy + dx)*2 + l
# B: R_B[pq32, dy, dx, l] same formula with pq32//8 in 0..3 (gi-pair groups).
def _mk_R(npq):
    pq = np.arange(npq)
    base = 16 * (pq // 8) + (pq % 8)
    return (
        (base[:, None, None, None]
         + 16 * np.arange(9)[None, :, None, None]
         + np.arange(9)[None, None, :, None]) * 2
        + np.arange(2)[None, None, None, :]
    ).reshape(1, 1, npq, 1, 162)


_RA = _mk_R(16)
_RB = _mk_R(32)
_R4 = _mk_R(64)


def kernel(t1: np.ndarray, t2: np.ndarray) -> np.ndarray:
    from concourse.bass_utils import run_bass_kernel_spmd

    global _compiled
    if _compiled is None:
        _compiled = _build()
    nc = _compiled

    t1 = np.asarray(t1, dtype=np.float32)
    t2 = np.asarray(t2, dtype=np.float32)
    res = run_bass_kernel_spmd(nc, _prep_inputs(t1, t2), list(range(8)))

    out = np.empty((B, D, H, W), dtype=np.float32)
    for k in range(8):
        b, xh = k // 2, k % 2
        xs = xh * WH
        raw = res.results[k]["outp"].astype(np.float32)
        # x = 32*slx + 8*lane + cc ; y = 32*st + 16*slh + (yl)
        # all regions normalize to [st, gi, ylq, cc, slh, slx, dy, dx, l]
        oc = np.empty((D, H, 4, 4, 8), dtype=np.float32)

        def pair_to_gi(g):  # [n,4g,32pq,slot,162] -> [n,8gi,2ylq,8cc,slh,slx,dy,dx,l]
            n = g.shape[0]
            return (g.reshape(n, 4, 2, 2, 8, 2, 4, 9, 9, 2)
                     .reshape(n, 8, 2, 8, 2, 4, 9, 9, 2))

        arrA = raw[:ASZ].reshape(3, 8, 16, NSLOT, GWH)
        gA = np.take_along_axis(arrA, _RA, axis=4)
        gA = gA.reshape(3, 8, 2, 8, 2, 4, 9, 9, 2)
        a3 = raw[ASZ + BSZ : ASZ + BSZ + A3SZ].reshape(1, 4, 32, NSLOT, GWB)
        gA3 = pair_to_gi(np.take_along_axis(a3, _RB, axis=4))
        gA = np.concatenate([gA, gA3], axis=0)
        gA = gA.transpose(6, 7, 0, 4, 1, 2, 5, 8, 3)  # [dy,dx,st,slh,gi,ylq,slx,l,cc]
        oc[:, :, :, 0:2, :] = gA.reshape(D, H, 4, 2, 8)

        arrB = raw[ASZ : ASZ + BSZ].reshape(3, 4, 32, NSLOT, GWB)
        gB = pair_to_gi(np.take_along_axis(arrB, _RB, axis=4))
        b3 = raw[ASZ + BSZ + A3SZ :].reshape(1, 2, 64, NSLOT, GW4)
        gB3 = np.take_along_axis(b3, _R4, axis=4)  # [1,g2,pq64,slot,162]
        # pq64 = yl8*8+cc, gi = 4*g2 + yl8//2, ylq = yl8%2
        gB3 = (gB3.reshape(1, 2, 4, 2, 8, 2, 4, 9, 9, 2)
                  .reshape(1, 8, 2, 8, 2, 4, 9, 9, 2))
        gB = np.concatenate([gB, gB3], axis=0)
        gB = gB.transpose(6, 7, 0, 4, 1, 2, 5, 8, 3)
        oc[:, :, :, 2:4, :] = gB.reshape(D, H, 4, 2, 8)
        out[b, :, :, xs : xs + WH] = oc.reshape(D, H, WH)
    return out


# revision 41
# speedup vs baseline: 1.0396x; 1.0381x over previous
"""PWC-Net local correlation (MD=4, 81 displacements) on 8 Trainium2 cores.

Problem: t1, t2: [B=4, C=128, H=128, W=256] fp32
  out[b, d, y, x] = mean_c t1[b,c,y,x] * t2pad[b,c,y+dy,x+dx],  d = (dy+4)*9+(dx+4)

Sharding: 8 cores = B(4) x W-half(2); inputs pre-sliced/padded/bf16-cast on
host (t1 pre-scaled by 1/C so the gram is already the mean).

v5: 2D-patch grams (16x8 pixels) instead of column grams -- 3.2x less PE and
evacuation work per pixel. Per patch (yb, xb): one bf16 matmul
  G[p, j] = sum_c t1[c, p] * t2p[c, window j],  p = yl*8+cc (128 pixels),
  j = wr*16 + wc over the 24x16 halo window (384 cols, one PSUM bank).
Useful entries: out[p, dy, dx] = G[p, 16*yl + cc + 16*dy + dx].

Four x-adjacent patches (xb = 4*xg+lane) form a qgroup. Hard-won structural
constraints (this + prior session evidence):
  - Tile serializes two readers of one PSUM tile cross-engine (ACT->DVE
    chain), and two writers of one SBUF tile -- so lanes 0,1 go to psP
    (read by ACT only) and lanes 2,3 to psQ (DVE only), each engine
    pair-interleaving its half into its own gsb tile:
    gsbA[p, slot*768 + j*2 + lane] (ACT), gsbB same (DVE).
  - composite AP dims crossing partitions+columns are illegal ("illegal
    partition step"), so per-octet rebased dumps are impossible; dumps move
    partition-uniform windows per 16-partition group gi (yl pair):
    j in [32*gi, 32*gi+160), flat cols [64*gi, 64*gi+320) of every slot,
    640B-contiguous runs, straight to output DRAM (1.98x amplification).
    Host does the final 81-of-640 gather (junk discard + (d,y,x) transpose);
    all arithmetic stays on device.
  - A-half dumps ride HWDGE (632ns/instr, shared); B-half dumps ride the
    otherwise-idle Pool/SWDGE path as gi-PAIR windows (32 partitions,
    [128g, +384), amp 2.37) -- 4 instead of 8 SWDGE issues per supertile so
    the last supertile's ~1us/instr Pool drain doesn't tail the run.
  - matmul PSUM outputs must stay inside one 2KB bank (512-aligned chunks);
    a strided interleaving matmul out AP silently corrupts.
  - matmul stationary (lhsT) APs allow one free dim -> t1 is pre-arranged
    patch-major on host so each patch's 128 pixels are contiguous.
  - DRAM->DRAM skewed DMA: fatal on hardware (NRT_EXEC_UNIT_UNRECOVERABLE).
  - t2 y-halo rows are NOT re-loaded from DRAM: slab q rows [0,8) come from
    slab q-1 rows [32,40) via a DVE SBUF->SBUF bf16 copy; all-pad rows are
    memset on-chip. Saves ~1.4MB HBM per core.
Steady state is DMA_ENGINES-bound: ~14.4MB/core of HBM traffic (8.7 in,
5.8 out at ~2x junk amplification -- the price of partition-uniform dump
windows) against ~360GB/s. Engines sit at 50-65%: ACT 28us, DVE 32us,
PE 24us, Pool 18us, HWDGE 26us.
v1 (column grams): 73.3us HW / 141us cost model. v5: 47.6us cost model.
"""

import numpy as np
import ml_dtypes

B, C, H, W = 4, 128, 128, 256
MD = 4
D = (2 * MD + 1) ** 2  # 81
WH = W // 2  # 128 columns per core
YP2 = H + 2 * MD  # 136 padded t2 rows
XP2 = WH + 2 * MD  # 136 padded t2 cols
WIN = 384  # 24x16 window cols per patch gram
LANES = 4  # patches per qgroup (2 per engine half)
HSLOT = 2 * WIN  # 768 gsb cols per slot in each half tile
NSLOT = 8  # slots per supertile (= 2 y-blocks x 4 x-groups)
GWH = 320  # per-gi dump window: 160 j x 2 lanes
GWB = 384  # gi-pair dump window: 192 j x 2 lanes
GW4 = 512  # 64-partition (4-gi) dump window: 256 j x 2 lanes
ASZ = 3 * 8 * 16 * NSLOT * GWH  # A region st 0-2 (unpaired gi dumps)
BSZ = 3 * 4 * 32 * NSLOT * GWB  # B region st 0-2 (gi-paired dumps)
A3SZ = 4 * 32 * NSLOT * GWB  # A region st 3: gi-paired, 4 HWDGE issues
B3SZ = 2 * 64 * NSLOT * GW4  # B region st 3: 2 Pool issues -> short tail
_compiled = None


def _build(reps=None):
    """Build the per-core program. reps=None: single pass. reps=R: wrap the
    compute in a hardware For loop (identical output each rep) for wall-clock
    benchmarking through the noisy RPC dispatch floor."""
    import concourse.bacc as bacc
    import concourse.bass as bass
    import concourse.mybir as mybir
    import concourse.tile as tile

    bf = mybir.dt.bfloat16
    nc = bacc.Bacc("TRN2", target_bir_lowering=False, debug=False, num_devices=8)
    t1s = nc.dram_tensor("t1s", [C, H * WH], bf, kind="ExternalInput").ap()
    t2s = nc.dram_tensor("t2s", [C, 4 * 40 * XP2], bf, kind="ExternalInput").ap()
    outp = nc.dram_tensor("outp", [ASZ + BSZ + A3SZ + B3SZ], bf,
                          kind="ExternalOutput").ap()

    with tile.TileContext(nc) as tc:
        with (
            tc.tile_pool(name="inputs", bufs=1) as inp,
            tc.tile_pool(name="gpool", bufs=3) as gpool,
            tc.tile_pool(name="gpool3", bufs=2) as gpool3,
            tc.tile_pool(name="psA", bufs=2, space="PSUM") as ppa,
            tc.tile_pool(name="psB", bufs=2, space="PSUM") as ppb,
        ):
            # one slab per supertile (yb pair): t1 4096 px patch-major,
            # t2 rows [32q-4, 32q+36) pre-padded on host
            t1t = [inp.tile([C, 32 * WH], bf, name=f"t1t_{q}") for q in range(4)]
            t2t = [inp.tile([C, 40 * XP2], bf, name=f"t2t_{q}") for q in range(4)]
            S1 = t1t[0].tensor.shape[-1]
            S2 = t2t[0].tensor.shape[-1]

            def load_slab(q, split):
                """Issue slab q's loads; split=True stages the pieces the
                first qgroups need in front (t1 1024-px mini, t2 24 rows).
                Slabs q>=1 load only rows [8,40) from DRAM; rows [0,8) (the
                8-row halo, = slab q-1's rows [32,40)) are filled by a cheap
                DVE SBUF->SBUF copy (bf16 4x mode), saving 0.84MB of HBM."""
                if split:
                    # rows [0,4) of slab 0 are all-pad: memset, load [4,24)
                    nc.vector.memset(
                        bass.AP(t2t[q].tensor, 0, [[S2, C], [1, 4 * XP2]]), 0.0
                    )
                    nc.sync.dma_start(
                        bass.AP(t1t[q].tensor, 0, [[S1, C], [1, 1024]]),
                        bass.AP(t1s.tensor, 32 * WH * q, [[H * WH, C], [1, 1024]]),
                    )
                    nc.sync.dma_start(
                        bass.AP(t2t[q].tensor, 4 * XP2, [[S2, C], [1, 20 * XP2]]),
                        bass.AP(t2s.tensor, 40 * XP2 * q + 4 * XP2,
                                [[4 * 40 * XP2, C], [1, 20 * XP2]]),
                    )
                    nc.sync.dma_start(
                        bass.AP(t1t[q].tensor, 1024, [[S1, C], [1, 32 * WH - 1024]]),
                        bass.AP(t1s.tensor, 32 * WH * q + 1024,
                                [[H * WH, C], [1, 32 * WH - 1024]]),
                    )
                    nc.sync.dma_start(
                        bass.AP(t2t[q].tensor, 24 * XP2, [[S2, C], [1, 16 * XP2]]),
                        bass.AP(t2s.tensor, 40 * XP2 * q + 24 * XP2,
                                [[4 * 40 * XP2, C], [1, 16 * XP2]]),
                    )
                else:
                    nc.sync.dma_start(
                        t1t[q][:],
                        bass.AP(t1s.tensor, 32 * WH * q, [[H * WH, C], [1, 32 * WH]]),
                    )
                    # rows [36,40) of slab 3 are all-pad: memset instead
                    rows = 28 if q == 3 else 32
                    nc.sync.dma_start(
                        bass.AP(t2t[q].tensor, 8 * XP2, [[S2, C], [1, rows * XP2]]),
                        bass.AP(t2s.tensor, 40 * XP2 * q + 8 * XP2,
                                [[4 * 40 * XP2, C], [1, rows * XP2]]),
                    )
                    if q == 3:
                        nc.vector.memset(
                            bass.AP(t2t[q].tensor, 36 * XP2, [[S2, C], [1, 4 * XP2]]),
                            0.0,
                        )
                    nc.vector.tensor_copy(
                        bass.AP(t2t[q].tensor, 0, [[S2, C], [1, 8 * XP2]]),
                        bass.AP(t2t[q - 1].tensor, 32 * XP2, [[S2, C], [1, 8 * XP2]]),
                    )

            load_slab(0, split=True)
            load_slab(1, split=False)

            def qgroup(st, ybh, xg, gsbA, gsbB, SgA, SgB, sl):
                        psP = ppa.tile([128, 1024], mybir.dt.float32, name="psP")
                        psQ = ppb.tile([128, 1024], mybir.dt.float32, name="psQ")
                        SpP = psP.tensor.shape[-1]
                        SpQ = psQ.tensor.shape[-1]
                        for lane in range(LANES):
                            xb = LANES * xg + lane
                            lhsT = bass.AP(
                                t1t[st].tensor, (ybh * 16 + xb) * 128,
                                [[S1, C], [1, 128]],
                            )
                            rhs = bass.AP(
                                t2t[st].tensor, (16 * ybh) * XP2 + 8 * xb,
                                [[S2, C], [XP2, 24], [1, 16]],
                            )
                            pst, Spt = (psP, SpP) if lane < 2 else (psQ, SpQ)
                            nc.tensor.matmul(
                                bass.AP(pst.tensor, 512 * (lane % 2),
                                        [[Spt, 128], [1, WIN]]),
                                lhsT, rhs, start=True, stop=True,
                            )
                        # pair-interleaving evacuation per engine half:
                        # gsbX[p, sl*768 + j*2 + lane%2]
                        nc.scalar.copy(
                            bass.AP(gsbA.tensor, HSLOT * sl,
                                    [[SgA, 128], [2, WIN], [1, 2]]),
                            bass.AP(psP.tensor, 0, [[SpP, 128], [1, WIN], [512, 2]]),
                        )
                        nc.vector.tensor_copy(
                            bass.AP(gsbB.tensor, HSLOT * sl,
                                    [[SgB, 128], [2, WIN], [1, 2]]),
                            bass.AP(psQ.tensor, 0, [[SpQ, 128], [1, WIN], [512, 2]]),
                        )

            def body(_iv=None):
                # st 0-2: 8-slot supertiles. dumps: A-half on HWDGE (SP), per
                # 16-partition group gi (yl pair), window cols [64gi, +320)
                # per slot; B-half on Pool/SWDGE as 32-partition gi-PAIRS
                # ([128g, +384)) -- 4 instead of 8 ~1us SWDGE issues
                # (unpaired-B measured slower: Pool issue rate starves it).
                for st in range(3):
                    if st + 2 < 4:
                        load_slab(st + 2, split=False)
                    gsbA = gpool.tile([C, NSLOT * HSLOT], bf, name="gsbA")
                    gsbB = gpool.tile([C, NSLOT * HSLOT], bf, name="gsbB")
                    SgA = gsbA.tensor.shape[-1]
                    SgB = gsbB.tensor.shape[-1]
                    for sl in range(NSLOT):
                        qgroup(st, sl // 4, sl % 4, gsbA, gsbB, SgA, SgB, sl)
                    for gi in range(8):
                        nc.sync.dma_start(
                            bass.AP(outp.tensor,
                                    st * (8 * 16 * NSLOT * GWH)
                                    + gi * (16 * NSLOT * GWH),
                                    [[NSLOT * GWH, 16], [GWH, NSLOT], [1, GWH]]),
                            bass.AP(gsbA.tensor, (16 * gi) * SgA + 64 * gi,
                                    [[SgA, 16], [HSLOT, NSLOT], [1, GWH]]),
                        )
                    for g in range(4):
                        nc.gpsimd.dma_start(
                            bass.AP(outp.tensor,
                                    ASZ + st * (4 * 32 * NSLOT * GWB)
                                    + g * (32 * NSLOT * GWB),
                                    [[NSLOT * GWB, 32], [GWB, NSLOT], [1, GWB]]),
                            bass.AP(gsbB.tensor, (32 * g) * SgB + 128 * g,
                                    [[SgB, 32], [HSLOT, NSLOT], [1, GWB]]),
                        )
                # st 3 runs as TWO half-supertiles (one y-block, 4 slots
                # each) so the first half's dumps drain while the second
                # half computes; only a half-sized drain tails the run.
                # Per half: A paired (4 HWDGE issues), B as two
                # 64-partition windows (2 Pool issues, j in [128g2, +256)).
                for half in range(2):
                    gsbA = gpool3.tile([C, 4 * HSLOT], bf, name="gsbA3")
                    gsbB = gpool3.tile([C, 4 * HSLOT], bf, name="gsbB3")
                    SgA = gsbA.tensor.shape[-1]
                    SgB = gsbB.tensor.shape[-1]
                    for xg in range(4):
                        qgroup(3, half, xg, gsbA, gsbB, SgA, SgB, xg)
                    for g in range(4):
                        nc.sync.dma_start(
                            bass.AP(outp.tensor,
                                    ASZ + BSZ + half * (4 * 32 * 4 * GWB)
                                    + g * (32 * 4 * GWB),
                                    [[4 * GWB, 32], [GWB, 4], [1, GWB]]),
                            bass.AP(gsbA.tensor, (32 * g) * SgA + 128 * g,
                                    [[SgA, 32], [HSLOT, 4], [1, GWB]]),
                        )
                    for g2 in range(2):
                        nc.gpsimd.dma_start(
                            bass.AP(outp.tensor,
                                    ASZ + BSZ + A3SZ + half * (2 * 64 * 4 * GW4)
                                    + g2 * (64 * 4 * GW4),
                                    [[4 * GW4, 64], [GW4, 4], [1, GW4]]),
                            bass.AP(gsbB.tensor, (64 * g2) * SgB + 256 * g2,
                                    [[SgB, 64], [HSLOT, 4], [1, GW4]]),
                        )

            if reps is None:
                body()
            else:
                with tc.For_i(0, reps, 1) as iv:
                    body(iv)

    nc.compile()
    return nc


def _prep_inputs(t1, t2):
    bf16 = ml_dtypes.bfloat16
    in_maps = []
    for k in range(8):
        b, xh = k // 2, k % 2
        xs = xh * WH
        t1c = (t1[b, :, :, xs : xs + WH] * (1.0 / C)).astype(bf16)
        # patch-major: [c, ((yb*16+xb)*16 + yl)*8 + cc], yb in 0..7, xb in
        # 0..15 (matmul stationary APs allow one free dim only)
        t1blk = np.ascontiguousarray(
            t1c.reshape(C, 8, 16, 16, 8).transpose(0, 1, 3, 2, 4)
        ).reshape(C, H * WH)
        t2p = np.zeros((C, YP2, XP2), dtype=bf16)
        lo, hi = max(0, xs - MD), min(W, xs + WH + MD)
        t2p[:, MD : MD + H, lo - (xs - MD) : hi - (xs - MD)] = t2[b, :, :, lo:hi].astype(bf16)
        # overlapping 40-row slabs per supertile (yb pair)
        t2blk = np.concatenate(
            [t2p[:, 32 * q : 32 * q + 40, :].reshape(C, 40 * XP2) for q in range(4)],
            axis=1,
        )
        in_maps.append({"t1s": t1blk, "t2s": t2blk})
    return in_maps


# host gathers. A: R_A[pq16, dy, dx, l] = (16*(pq//8) + pq%8 + 16dy + dx)*2 + l
# B: R_B[pq32, dy, dx, l] same formula with pq32//8 in 0..3 (gi-pair groups).
def _mk_R(npq):
    pq = np.arange(npq)
    base = 16 * (pq // 8) + (pq % 8)
    return (
        (base[:, None, None, None]
         + 16 * np.arange(9)[None, :, None, None]
         + np.arange(9)[None, None, :, None]) * 2
        + np.arange(2)[None, None, None, :]
    ).reshape(1, 1, npq, 1, 162)


_RA = _mk_R(16)
_RB = _mk_R(32)
_R4 = _mk_R(64)


def kernel(t1: np.ndarray, t2: np.ndarray) -> np.ndarray:
    from concourse.bass_utils import run_bass_kernel_spmd

    global _compiled
    if _compiled is None:
        _compiled = _build()
    nc = _compiled

    t1 = np.asarray(t1, dtype=np.float32)
    t2 = np.asarray(t2, dtype=np.float32)
    res = run_bass_kernel_spmd(nc, _prep_inputs(t1, t2), list(range(8)))

    out = np.empty((B, D, H, W), dtype=np.float32)
    for k in range(8):
        b, xh = k // 2, k % 2
        xs = xh * WH
        raw = res.results[k]["outp"].astype(np.float32)
        # x = 32*slx + 8*lane + cc ; y = 32*st + 16*slh + (yl)
        # all regions normalize to [st, gi, ylq, cc, slh, slx, dy, dx, l]
        oc = np.empty((D, H, 4, 4, 8), dtype=np.float32)

        def pair_to_gi(g):  # [n,4g,32pq,slot,162] -> [n,8gi,2ylq,8cc,slh,slx,dy,dx,l]
            n = g.shape[0]
            return (g.reshape(n, 4, 2, 2, 8, 2, 4, 9, 9, 2)
                     .reshape(n, 8, 2, 8, 2, 4, 9, 9, 2))

        arrA = raw[:ASZ].reshape(3, 8, 16, NSLOT, GWH)
        gA = np.take_along_axis(arrA, _RA, axis=4)
        gA = gA.reshape(3, 8, 2, 8, 2, 4, 9, 9, 2)
        a3 = raw[ASZ + BSZ : ASZ + BSZ + A3SZ].reshape(2, 4, 32, 4, GWB)
        gA3 = np.take_along_axis(a3, _RB, axis=4)  # [half,g,pq32,slot4,162]
        gA3 = (gA3.reshape(2, 4, 2, 2, 8, 4, 9, 9, 2)
                  .transpose(1, 2, 3, 4, 0, 5, 6, 7, 8)
                  .reshape(1, 8, 2, 8, 2, 4, 9, 9, 2))  # slh = half
        gA = np.concatenate([gA, gA3], axis=0)
        gA = gA.transpose(6, 7, 0, 4, 1, 2, 5, 8, 3)  # [dy,dx,st,slh,gi,ylq,slx,l,cc]
        oc[:, :, :, 0:2, :] = gA.reshape(D, H, 4, 2, 8)

        arrB = raw[ASZ : ASZ + BSZ].reshape(3, 4, 32, NSLOT, GWB)
        gB = pair_to_gi(np.take_along_axis(arrB, _RB, axis=4))
        b3 = raw[ASZ + BSZ + A3SZ :].reshape(2, 2, 64, 4, GW4)
        gB3 = np.take_along_axis(b3, _R4, axis=4)  # [half,g2,pq64,slot4,162]
        # pq64 = yl8*8+cc, gi = 4*g2 + yl8//2, ylq = yl8%2
        gB3 = (gB3.reshape(2, 2, 4, 2, 8, 4, 9, 9, 2)
                  .transpose(1, 2, 3, 4, 0, 5, 6, 7, 8)
                  .reshape(1, 8, 2, 8, 2, 4, 9, 9, 2))
        gB = np.concatenate([gB, gB3], axis=0)
        gB = gB.transpose(6, 7, 0, 4, 1, 2, 5, 8, 3)
        oc[:, :, :, 2:4, :] = gB.reshape(D, H, 4, 2, 8)
        out[b, :, :, xs : xs + WH] = oc.reshape(D, H, WH)
    return out
